# revision 27
# baseline (speedup 1.0000x reference)
"""4-bit column-block-quantized linear (ColBlockQuantizedLinear) on 8 TRN2 cores.

Math:  out[b,o] = scales[o] * (sum_i inp[b,i]*wq[o,i] - zeros[o]*rowsum[b])
with packed bytes q[j,o] (j = i//2): low nibble l = wq[o,2j], high nibble
h = wq[o,2j+1].  Identity: sum_j a_j*l_j + b_j*h_j = sum_j a_j*q_j + c_j*h_j
with c = b - 16a, q = 16h + l.

Device scheme (fp16 bit-trick): the fp16 bit pattern 0x5800|x encodes the
value 128 + x/8 EXACTLY for any 8-bit x.  So each weight stream is ONE
dual-op DVE tensor_scalar pass over the packed u16 data:
    Qlo = (q16 & 0x00FF) | 0x5800   -> 128 + q_lo/8      (pairs with 8a)
    Qhi = (q16 >>  8)    | 0x5800   -> 128 + q_hi/8      (pairs with 8a)
    Hlo = (q16 & 0x00F0) | 0x5800   -> 128 + 2*h_lo      (pairs with c/2)
    Hhi = (q16 >> 12)    | 0x5800   -> 128 + h_hi/8      (pairs with 8c)
The +128 offsets cancel exactly against rank-1 rows built from the SAME
fp16-rounded stationaries, folded with -zeros*rowsum into a K=9 fp16 hi/lo
correction matmul issued LAST (start=True sits on the first stream matmuls;
corrections run warm and overlap the per-block scale/DMA tail).  Stationary
activation factors are single fp16.  Scales are applied on-device by
per-psum-block DVE tensor_tensor multiplies.

DMA economics (measured): a DMA instruction costs ~1.5-2us of queue overhead
plus ~one packet per SBUF partition row (~20-27ns each, FIFO per queue), so
the weight load is split into exactly FIVE single DMAs, one per engine queue
(sync/scalar/gpsimd/vector/tensor), each first-and-alone on its queue, sized
in need order.  kt0 and its stationaries ride the first sync DMA (fusion is
free - packets are per row); the remaining stationaries ride the scalar DMA.
All weight tiles stay resident in SBUF (~24KB/partition).  DVE unpack runs
in kt-pair slices over the resident tiles.  Seven zero matmuls on a memset
tile warm the PE (HAM un-throttle needs ~3.4us of sustained activity) so
the stream matmuls start at 2.4GHz.

Sharding: column-parallel over out_features (1376 rows/core), inputs
replicated; per-core output [16,1376] gathered on host.
"""

import numpy as np

B = 16
I = 4096
O = 11008
NCORES = 8
OS = O // NCORES          # 1376 out-features per core
HOS = OS // 2             # 688 packed u16 columns
HALF = I // 2             # 2048 packed (contraction) rows
KT = HALF // 128          # 16 contraction tiles
KC = 9                    # correction matmul contraction size
NDUMMY = 6                # PE warmup matmuls
SPL = 512                 # kt0 column split (psum-block boundary)
NS0 = 4                   # kt tiles whose stationaries ride DMA d0
# weight pair-DMAs after d0/d1: (first kt, n kt) -> round-robin queues in
# need order (gpsimd, sync, scalar, gpsimd, sync, scalar, gpsimd)
PAIRS = [(3, 2), (5, 2), (7, 2), (9, 2), (11, 2), (13, 2), (15, 1)]
# DVE/compute sub-chunks: (first kt, n kt)
CCHUNKS = [(1, 2), (3, 2), (5, 2), (7, 2), (9, 2), (11, 2), (13, 2), (15, 1)]
# psum o-blocks (each within one 688-column half, <=512 cols per fp32 bank)
BLKS = [(0, 512), (512, 176), (688, 512), (1200, 176)]

F16 = np.float16

_CACHE = {}


def _build_program():
    import concourse.bacc as bacc
    import concourse.mybir as mybir
    import concourse.tile as tile

    dt = mybir.dt
    op = mybir.AluOpType
    nc = bacc.Bacc("TRN2", target_bir_lowering=False)

    # d0 = [kt0 | stat(kt0..NS0)]; d1 = [stat(rest) | kt1-2]; d2..d4 = kt runs
    d0 = nc.dram_tensor("d0", [128, HOS + NS0 * 48], dt.uint16, kind="ExternalInput")
    d1 = nc.dram_tensor(
        "d1", [128, (KT - NS0) * 48 + 2 * HOS], dt.uint16, kind="ExternalInput"
    )
    q = nc.dram_tensor("q", [128, 13 * HOS], dt.uint16, kind="ExternalInput")
    csc = nc.dram_tensor(
        "csc", [B, (16 + OS) + 2 * OS], dt.uint16, kind="ExternalInput"
    )
    out = nc.dram_tensor("out", [B, OS], dt.float32, kind="ExternalOutput")

    with tile.TileContext(nc) as tc:
        with (
            tc.tile_pool(name="consts", bufs=1) as cpool,
            tc.tile_pool(name="wp", bufs=3) as wpool,
            tc.tile_pool(name="op", bufs=1) as opool,
            tc.tile_pool(name="ps", bufs=1, space="PSUM") as pspool,
        ):
            # PE warmup: zero matmuls while DMAs/DVE fill the pipeline
            dummy = cpool.tile([128, 512], dt.float16, name="dummy")
            ps_w = pspool.tile([16, 512], dt.float32, name="ps_w")
            nc.vector.memset(dummy, 0.0)
            for _ in range(NDUMMY):
                nc.tensor.matmul(
                    ps_w, dummy[:, 0:16], dummy, start=True, stop=True,
                    skip_group_check=True,
                )

            t0 = cpool.tile([128, HOS + NS0 * 48], dt.uint16, name="t0")
            nc.sync.dma_start(t0, d0[:, :])
            t1 = cpool.tile(
                [128, (KT - NS0) * 48 + 2 * HOS], dt.uint16, name="t1"
            )
            nc.scalar.dma_start(t1, d1[:, :])
            QRR = [nc.gpsimd, nc.sync, nc.scalar]
            ptile = {}
            for pi, (k0, cw) in enumerate(PAIRS):
                pt = cpool.tile([128, cw * HOS], dt.uint16, name=f"pt{pi}")
                QRR[pi % 3].dma_start(
                    pt, q[:, (k0 - 3) * HOS : (k0 - 3 + cw) * HOS]
                )
                ptile[k0] = pt

            csc_sb = cpool.tile([B, 16 + OS + 2 * OS], dt.uint16, name="csc_sb")
            nc.sync.dma_start(csc_sb, csc[:, :])
            corr16 = csc_sb.bitcast(dt.float16)
            corrL_sb = corr16[0:KC, 0:16]
            corrR_sb = corr16[0:KC, 16 : 16 + OS]
            sc_sb = csc_sb.bitcast(dt.float32)[:, (16 + OS) // 2 : (16 + OS) // 2 + OS]

            st0_16 = t0.bitcast(dt.float16)  # stat kt0..NS0-1 at cols HOS+
            st1_16 = t1.bitcast(dt.float16)  # stat NS0..KT-1 at cols 0+

            def stat_slices(kt):
                if kt < NS0:
                    sb = HOS + kt * 48
                    src = st0_16
                else:
                    sb = (kt - NS0) * 48
                    src = st1_16
                return (
                    src[:, sb : sb + 16],
                    src[:, sb + 16 : sb + 32],
                    src[:, sb + 32 : sb + 48],
                )

            # source tile + column offset for each kt's packed data
            def kt_src(kt):
                if kt == 0:
                    return t0, 0
                if kt <= 2:
                    return t1, (KT - NS0) * 48 + (kt - 1) * HOS
                return ptile[kt], 0

            psums = [
                pspool.tile([B, n], dt.float32, name=f"ps{i}")
                for i, (s, n) in enumerate(BLKS)
            ]

            def unpack(dst4, src_ap):
                nc.vector.tensor_scalar(
                    dst4[0], src_ap, 0x00FF, 0x5800, op.bitwise_and, op.bitwise_or
                )
                nc.vector.tensor_scalar(
                    dst4[1], src_ap, 8, 0x5800, op.logical_shift_right, op.bitwise_or
                )
                nc.vector.tensor_scalar(
                    dst4[2], src_ap, 0x00F0, 0x5800, op.bitwise_and, op.bitwise_or
                )
                nc.vector.tensor_scalar(
                    dst4[3], src_ap, 12, 0x5800, op.logical_shift_right, op.bitwise_or
                )

            def kt_matmuls(kt, st4, off, first, last=False):
                sq, shlo, shhi = stat_slices(kt)
                qlo16 = st4[0].bitcast(dt.float16)
                qhi16 = st4[1].bitcast(dt.float16)
                hlo16 = st4[2].bitcast(dt.float16)
                hhi16 = st4[3].bitcast(dt.float16)
                for i, (s, n) in enumerate(BLKS):
                    if s < HOS:
                        a, b_ = off + s, off + s + n
                        nc.tensor.matmul(
                            psums[i], sq, qlo16[:, a:b_], start=first, stop=False
                        )
                        nc.tensor.matmul(
                            psums[i], shlo, hlo16[:, a:b_], start=False, stop=last
                        )
                    else:
                        a, b_ = off + s - HOS, off + s - HOS + n
                        nc.tensor.matmul(
                            psums[i], sq, qhi16[:, a:b_], start=first, stop=False
                        )
                        nc.tensor.matmul(
                            psums[i], shhi, hhi16[:, a:b_], start=False, stop=last
                        )

            # --- kt0 in two column pieces aligned to psum-block boundaries:
            # piece A (cols 0..511) feeds blocks 0,2; piece B feeds 1,3
            nB = HOS - SPL
            stA = [cpool.tile([128, SPL], dt.uint16, name=f"sA{k}") for k in range(4)]
            stB = [cpool.tile([128, nB], dt.uint16, name=f"sB{k}") for k in range(4)]
            unpack(stA, t0[:, 0:SPL])
            unpack(stB, t0[:, SPL:HOS])
            sq0, shlo0, shhi0 = stat_slices(0)
            nc.tensor.matmul(
                psums[0], sq0, stA[0].bitcast(dt.float16), start=True, stop=False
            )
            nc.tensor.matmul(
                psums[0], shlo0, stA[2].bitcast(dt.float16), start=False, stop=False
            )
            nc.tensor.matmul(
                psums[2], sq0, stA[1].bitcast(dt.float16), start=True, stop=False
            )
            nc.tensor.matmul(
                psums[2], shhi0, stA[3].bitcast(dt.float16), start=False, stop=False
            )
            nc.tensor.matmul(
                psums[1], sq0, stB[0].bitcast(dt.float16), start=True, stop=False
            )
            nc.tensor.matmul(
                psums[1], shlo0, stB[2].bitcast(dt.float16), start=False, stop=False
            )
            nc.tensor.matmul(
                psums[3], sq0, stB[1].bitcast(dt.float16), start=True, stop=False
            )
            nc.tensor.matmul(
                psums[3], shhi0, stB[3].bitcast(dt.float16), start=False, stop=False
            )

            # corrections early (off the tail; accumulation order is free):
            # -128*sum(coef) offsets and -zeros*rowsum
            for i, (s, n) in enumerate(BLKS):
                nc.tensor.matmul(
                    psums[i], corrL_sb, corrR_sb[:, s : s + n],
                    start=False, stop=False,
                )

            # --- kt1..15 in pair sub-chunks over the resident tiles
            for pi, (k0, cw) in enumerate(CCHUNKS):
                w = cw * HOS
                src, soff = kt_src(k0)
                st4 = [
                    wpool.tile([128, w], dt.uint16, name=f"s{pi}_{k}", tag=f"s{k}_{cw}")
                    for k in range(4)
                ]
                unpack(st4, src[:, soff : soff + w])
                for h in range(cw):
                    kt = k0 + h
                    kt_matmuls(kt, st4, h * HOS, False, last=(kt == KT - 1))

            o = opool.tile([B, OS], dt.float32, name="o")
            for i, (s, n) in enumerate(BLKS):
                nc.vector.tensor_tensor(
                    o[:, s : s + n], psums[i], sc_sb[:, s : s + n], op.mult
                )
            nc.gpsimd.dma_start(out[:, 0:HOS], o[:, 0:HOS])
            nc.scalar.dma_start(out[:, HOS:OS], o[:, HOS:OS])

    nc.finalize()
    return nc


def _get_program():
    if "nc" not in _CACHE:
        _CACHE["nc"] = _build_program()
    return _CACHE["nc"]


def _split_hi_lo(x64):
    hi = x64.astype(F16)
    lo = (x64 - hi.astype(np.float64)).astype(F16)
    return hi, lo


def _host_prep(inp, quant_weight, scales, zeros):
    """Per-core input maps: layout/precision prep only, no O(O*I) math."""
    inp64 = np.asarray(inp, dtype=np.float64)
    a = inp64[:, 0::2].T  # [HALF, B] even-i activations (pair with l / q)
    b = inp64[:, 1::2].T  # [HALF, B] odd-i activations (pair with h)
    c = b - 16.0 * a

    sq = (8.0 * a).astype(F16)      # [HALF, B]
    shlo = (c / 2.0).astype(F16)
    shhi = (8.0 * c).astype(F16)

    stat = np.zeros((128, KT * 48), dtype=F16)
    for kt in range(KT):
        rows = slice(kt * 128, (kt + 1) * 128)
        stat[:, kt * 48 : kt * 48 + 16] = sq[rows]
        stat[:, kt * 48 + 16 : kt * 48 + 32] = shlo[rows]
        stat[:, kt * 48 + 32 : kt * 48 + 48] = shhi[rows]

    # correction batch vectors from the ROUNDED stationaries (exact cancel)
    sum_sq = sq.astype(np.float64).sum(axis=0)      # [B]
    sum_shlo = shlo.astype(np.float64).sum(axis=0)
    sum_shhi = shhi.astype(np.float64).sum(axis=0)
    rowsum = inp64.sum(axis=1)                      # [B]
    sq_h, sq_l = _split_hi_lo(sum_sq)
    slo_h, slo_l = _split_hi_lo(sum_shlo)
    shi_h, shi_l = _split_hi_lo(sum_shhi)
    rs_h, rs_l = _split_hi_lo(rowsum)
    corrL = np.zeros((KC, 16), dtype=F16)
    corrL[0], corrL[1] = sq_h, sq_l
    corrL[2], corrL[3] = slo_h, slo_l
    corrL[4], corrL[5] = shi_h, shi_l
    corrL[6], corrL[7] = rs_h, rs_h
    corrL[8] = rs_l

    qw = np.asarray(quant_weight)
    scales = np.asarray(scales, dtype=np.float64).reshape(-1)
    zeros = np.asarray(zeros, dtype=np.float64).reshape(-1)

    stat_u16 = stat.view(np.uint16)
    in_maps = []
    for cidx in range(NCORES):
        rows = slice(cidx * OS, (cidx + 1) * OS)
        qc = qw[rows].astype(np.uint8).T  # [HALF, OS] natural columns
        # byte-pair columns (m, 688+m) -> uint16 elements
        qc2 = np.empty((HALF, OS), dtype=np.uint8)
        qc2[:, 0::2] = qc[:, :HOS]
        qc2[:, 1::2] = qc[:, HOS:]
        qu16 = np.ascontiguousarray(qc2).view(np.uint16)  # [HALF, HOS]
        # regroup rows: q_all[r, kt*HOS + m] = qu16[kt*128 + r, m]
        q_all = np.ascontiguousarray(
            qu16.reshape(KT, 128, HOS).transpose(1, 0, 2).reshape(128, KT * HOS)
        )
        d0_c = np.concatenate(
            [q_all[:, 0:HOS], stat_u16[:, 0 : NS0 * 48]], axis=1
        )
        d1_c = np.concatenate(
            [stat_u16[:, NS0 * 48 :], q_all[:, HOS : 3 * HOS]], axis=1
        )
        q_c = np.ascontiguousarray(q_all[:, 3 * HOS :])

        z = zeros[rows]
        z_h, z_l = _split_hi_lo(z)
        corr_c = np.zeros((KC, 16 + OS), dtype=F16)
        corr_c[:, 0:16] = corrL
        corrR = corr_c[:, 16:]
        corrR[0] = -128.0
        corrR[1] = -128.0
        corrR[2, :HOS] = -128.0
        corrR[3, :HOS] = -128.0
        corrR[4, HOS:] = -128.0
        corrR[5, HOS:] = -128.0
        corrR[6] = -z_h
        corrR[7] = -z_l
        corrR[8] = -z_h
        sc_c = np.broadcast_to(scales[rows].astype(np.float32), (B, OS))
        csc_c = np.zeros((B, 16 + OS + 2 * OS), dtype=np.uint16)
        csc_c[0:KC, 0 : 16 + OS] = corr_c.view(np.uint16)
        csc_c[:, 16 + OS :] = np.ascontiguousarray(sc_c).view(np.uint16)
        in_maps.append(
            {
                "d0": d0_c,
                "d1": d1_c,
                "q": q_c,
                "csc": csc_c,
            }
        )
    return in_maps


def kernel(inp, quant_weight, scales, zeros):
    from concourse.bass_utils import run_bass_kernel_spmd

    nc = _get_program()
    in_maps = _host_prep(inp, quant_weight, scales, zeros)
    res = run_bass_kernel_spmd(nc, in_maps, core_ids=list(range(NCORES)))
    out = np.concatenate(
        [res.results[c]["out"] for c in range(NCORES)], axis=1
    )
    return np.ascontiguousarray(out.astype(np.float32))


# revision 29
# speedup vs baseline: 1.1819x; 1.1819x over previous
"""4-bit column-block-quantized linear (ColBlockQuantizedLinear) on 8 TRN2 cores.

Math:  out[b,o] = scales[o] * (sum_i inp[b,i]*wq[o,i] - zeros[o]*rowsum[b])
with packed bytes q[j,o] (j = i//2): low nibble l = wq[o,2j], high nibble
h = wq[o,2j+1].  Identity: sum_j a_j*l_j + b_j*h_j = sum_j a_j*q_j + c_j*h_j
with c = b - 16a, q = 16h + l.

Device scheme (fp16 bit-trick): the fp16 bit pattern 0x5800|x encodes the
value 128 + x/8 EXACTLY for any 8-bit x.  So each weight stream is ONE
dual-op DVE tensor_scalar pass over the packed u16 data:
    Qlo = (q16 & 0x00FF) | 0x5800   -> 128 + q_lo/8      (pairs with 8a)
    Qhi = (q16 >>  8)    | 0x5800   -> 128 + q_hi/8      (pairs with 8a)
    Hlo = (q16 & 0x00F0) | 0x5800   -> 128 + 2*h_lo      (pairs with c/2)
    Hhi = (q16 >> 12)    | 0x5800   -> 128 + h_hi/8      (pairs with 8c)
The +128 offsets cancel exactly against rank-1 rows built from the SAME
fp16-rounded stationaries, folded with -zeros*rowsum into a K=9 fp16 hi/lo
correction matmul issued LAST (start=True sits on the first stream matmuls;
corrections run warm and overlap the per-block scale/DMA tail).  Stationary
activation factors are single fp16.  Scales are applied on-device by
per-psum-block DVE tensor_tensor multiplies.

DMA economics (measured): a DMA instruction costs ~1.5-2us of queue overhead
plus ~one packet per SBUF partition row (~20-27ns each, FIFO per queue), so
the weight load is split into exactly FIVE single DMAs, one per engine queue
(sync/scalar/gpsimd/vector/tensor), each first-and-alone on its queue, sized
in need order.  kt0 and its stationaries ride the first sync DMA (fusion is
free - packets are per row); the remaining stationaries ride the scalar DMA.
All weight tiles stay resident in SBUF (~24KB/partition).  DVE unpack runs
in kt-pair slices over the resident tiles.  Seven zero matmuls on a memset
tile warm the PE (HAM un-throttle needs ~3.4us of sustained activity) so
the stream matmuls start at 2.4GHz.

Sharding: column-parallel over out_features (1376 rows/core), inputs
replicated; per-core output [16,1376] gathered on host.
"""

import numpy as np

B = 16
I = 4096
O = 11008
NCORES = 8
OS = O // NCORES          # 1376 out-features per core
HOS = OS // 2             # 688 packed u16 columns
HALF = I // 2             # 2048 packed (contraction) rows
KT = HALF // 128          # 16 contraction tiles
KC = 9                    # correction matmul contraction size
NDUMMY = 6                # PE warmup matmuls
SPL = 512                 # kt0 column split (psum-block boundary)
NS0 = 4                   # kt tiles whose stationaries ride DMA d0
# weight pair-DMAs after d0/d1: (first kt, n kt) -> round-robin queues in
# need order (gpsimd, sync, scalar, gpsimd, sync, scalar, gpsimd)
PAIRS = [(3, 2), (5, 2), (7, 2), (9, 2), (11, 2), (13, 2), (15, 1)]
# DVE/compute sub-chunks: (first kt, n kt)
CCHUNKS = [(1, 2), (3, 2), (5, 2), (7, 2), (9, 2), (11, 2), (13, 2), (15, 1)]
# psum o-blocks (each within one 688-column half, <=512 cols per fp32 bank)
BLKS = [(0, 512), (512, 176), (688, 512), (1200, 176)]

F16 = np.float16

_CACHE = {}


def _build_program():
    import concourse.bacc as bacc
    import concourse.mybir as mybir
    import concourse.tile as tile

    dt = mybir.dt
    op = mybir.AluOpType
    nc = bacc.Bacc("TRN2", target_bir_lowering=False)

    # d0 = [kt0 | stat(kt0..NS0)]; d1 = [stat(rest) | kt1-2]; d2..d4 = kt runs
    d0 = nc.dram_tensor("d0", [128, HOS + NS0 * 48], dt.uint16, kind="ExternalInput")
    d1 = nc.dram_tensor(
        "d1", [128, (KT - NS0) * 48 + 2 * HOS], dt.uint16, kind="ExternalInput"
    )
    q = nc.dram_tensor("q", [128, 13 * HOS], dt.uint16, kind="ExternalInput")
    csc = nc.dram_tensor(
        "csc", [B, (16 + OS) + 2 * OS], dt.uint16, kind="ExternalInput"
    )
    out = nc.dram_tensor("out", [B, OS], dt.float32, kind="ExternalOutput")

    with tile.TileContext(nc) as tc:
        with (
            tc.tile_pool(name="consts", bufs=1) as cpool,
            tc.tile_pool(name="wp", bufs=3) as wpool,
            tc.tile_pool(name="op", bufs=1) as opool,
            tc.tile_pool(name="ps", bufs=1, space="PSUM") as pspool,
        ):
            # PE warmup: zero matmuls while DMAs/DVE fill the pipeline
            dummy = cpool.tile([128, 512], dt.float16, name="dummy")
            ps_w = pspool.tile([16, 512], dt.float32, name="ps_w")
            nc.vector.memset(dummy, 0.0)
            for _ in range(NDUMMY):
                nc.tensor.matmul(
                    ps_w, dummy[:, 0:16], dummy, start=True, stop=True,
                    skip_group_check=True,
                )

            t0 = cpool.tile([128, HOS + NS0 * 48], dt.uint16, name="t0")
            nc.sync.dma_start(t0, d0[:, :])
            t1 = cpool.tile(
                [128, (KT - NS0) * 48 + 2 * HOS], dt.uint16, name="t1"
            )
            nc.scalar.dma_start(t1, d1[:, :])
            csc_sb = cpool.tile([B, 16 + OS + 2 * OS], dt.uint16, name="csc_sb")
            nc.sync.dma_start(csc_sb, csc[:, :])

            QRR = [nc.gpsimd, nc.sync, nc.scalar]
            ptile = {}
            for pi, (k0, cw) in enumerate(PAIRS):
                pt = cpool.tile([128, cw * HOS], dt.uint16, name=f"pt{pi}")
                QRR[pi % 3].dma_start(
                    pt, q[:, (k0 - 3) * HOS : (k0 - 3 + cw) * HOS]
                )
                ptile[k0] = pt
            corr16 = csc_sb.bitcast(dt.float16)
            corrL_sb = corr16[0:KC, 0:16]
            corrR_sb = corr16[0:KC, 16 : 16 + OS]
            sc_sb = csc_sb.bitcast(dt.float32)[:, (16 + OS) // 2 : (16 + OS) // 2 + OS]

            st0_16 = t0.bitcast(dt.float16)  # stat kt0..NS0-1 at cols HOS+
            st1_16 = t1.bitcast(dt.float16)  # stat NS0..KT-1 at cols 0+

            def stat_slices(kt):
                if kt < NS0:
                    sb = HOS + kt * 48
                    src = st0_16
                else:
                    sb = (kt - NS0) * 48
                    src = st1_16
                return (
                    src[:, sb : sb + 16],
                    src[:, sb + 16 : sb + 32],
                    src[:, sb + 32 : sb + 48],
                )

            # source tile + column offset for each kt's packed data
            def kt_src(kt):
                if kt == 0:
                    return t0, 0
                if kt <= 2:
                    return t1, (KT - NS0) * 48 + (kt - 1) * HOS
                return ptile[kt], 0

            psums = [
                pspool.tile([B, n], dt.float32, name=f"ps{i}")
                for i, (s, n) in enumerate(BLKS)
            ]

            def unpack(dst4, src_ap):
                nc.vector.tensor_scalar(
                    dst4[0], src_ap, 0x00FF, 0x5800, op.bitwise_and, op.bitwise_or
                )
                nc.vector.tensor_scalar(
                    dst4[1], src_ap, 8, 0x5800, op.logical_shift_right, op.bitwise_or
                )
                nc.vector.tensor_scalar(
                    dst4[2], src_ap, 0x00F0, 0x5800, op.bitwise_and, op.bitwise_or
                )
                nc.vector.tensor_scalar(
                    dst4[3], src_ap, 12, 0x5800, op.logical_shift_right, op.bitwise_or
                )

            def kt_matmuls(kt, st4, off, first, last=False):
                sq, shlo, shhi = stat_slices(kt)
                qlo16 = st4[0].bitcast(dt.float16)
                qhi16 = st4[1].bitcast(dt.float16)
                hlo16 = st4[2].bitcast(dt.float16)
                hhi16 = st4[3].bitcast(dt.float16)
                for i, (s, n) in enumerate(BLKS):
                    if s < HOS:
                        a, b_ = off + s, off + s + n
                        nc.tensor.matmul(
                            psums[i], sq, qlo16[:, a:b_], start=first, stop=False
                        )
                        nc.tensor.matmul(
                            psums[i], shlo, hlo16[:, a:b_], start=False, stop=last
                        )
                    else:
                        a, b_ = off + s - HOS, off + s - HOS + n
                        nc.tensor.matmul(
                            psums[i], sq, qhi16[:, a:b_], start=first, stop=False
                        )
                        nc.tensor.matmul(
                            psums[i], shhi, hhi16[:, a:b_], start=False, stop=last
                        )

            # --- kt0 in two column pieces aligned to psum-block boundaries:
            # piece A (cols 0..511) feeds blocks 0,2; piece B feeds 1,3
            nB = HOS - SPL
            stA = [cpool.tile([128, SPL], dt.uint16, name=f"sA{k}") for k in range(4)]
            stB = [cpool.tile([128, nB], dt.uint16, name=f"sB{k}") for k in range(4)]
            unpack(stA, t0[:, 0:SPL])
            unpack(stB, t0[:, SPL:HOS])
            sq0, shlo0, shhi0 = stat_slices(0)
            nc.tensor.matmul(
                psums[0], sq0, stA[0].bitcast(dt.float16), start=True, stop=False
            )
            nc.tensor.matmul(
                psums[0], shlo0, stA[2].bitcast(dt.float16), start=False, stop=False
            )
            nc.tensor.matmul(
                psums[2], sq0, stA[1].bitcast(dt.float16), start=True, stop=False
            )
            nc.tensor.matmul(
                psums[2], shhi0, stA[3].bitcast(dt.float16), start=False, stop=False
            )
            nc.tensor.matmul(
                psums[1], sq0, stB[0].bitcast(dt.float16), start=True, stop=False
            )
            nc.tensor.matmul(
                psums[1], shlo0, stB[2].bitcast(dt.float16), start=False, stop=False
            )
            nc.tensor.matmul(
                psums[3], sq0, stB[1].bitcast(dt.float16), start=True, stop=False
            )
            nc.tensor.matmul(
                psums[3], shhi0, stB[3].bitcast(dt.float16), start=False, stop=False
            )

            # --- kt1..15 in pair sub-chunks over the resident tiles; the
            # corrections (-128*sum(coef), -zeros*rowsum) slot in mid-phase,
            # off the tail (accumulation order is free)
            for pi, (k0, cw) in enumerate(CCHUNKS):
                w = cw * HOS
                src, soff = kt_src(k0)
                st4 = [
                    wpool.tile([128, w], dt.uint16, name=f"s{pi}_{k}", tag=f"s{k}_{cw}")
                    for k in range(4)
                ]
                unpack(st4, src[:, soff : soff + w])
                for h in range(cw):
                    kt = k0 + h
                    kt_matmuls(kt, st4, h * HOS, False, last=(kt == KT - 1))
                if k0 == 5:
                    for i, (s, n) in enumerate(BLKS):
                        nc.tensor.matmul(
                            psums[i], corrL_sb, corrR_sb[:, s : s + n],
                            start=False, stop=False,
                        )

            o = opool.tile([B, OS], dt.float32, name="o")
            for i, (s, n) in enumerate(BLKS):
                nc.vector.tensor_tensor(
                    o[:, s : s + n], psums[i], sc_sb[:, s : s + n], op.mult
                )
            nc.gpsimd.dma_start(out[:, 0:HOS], o[:, 0:HOS])
            nc.scalar.dma_start(out[:, HOS:OS], o[:, HOS:OS])

    nc.finalize()
    return nc


def _get_program():
    if "nc" not in _CACHE:
        _CACHE["nc"] = _build_program()
    return _CACHE["nc"]


def _split_hi_lo(x64):
    hi = x64.astype(F16)
    lo = (x64 - hi.astype(np.float64)).astype(F16)
    return hi, lo


def _host_prep(inp, quant_weight, scales, zeros):
    """Per-core input maps: layout/precision prep only, no O(O*I) math."""
    inp64 = np.asarray(inp, dtype=np.float64)
    a = inp64[:, 0::2].T  # [HALF, B] even-i activations (pair with l / q)
    b = inp64[:, 1::2].T  # [HALF, B] odd-i activations (pair with h)
    c = b - 16.0 * a

    sq = (8.0 * a).astype(F16)      # [HALF, B]
    shlo = (c / 2.0).astype(F16)
    shhi = (8.0 * c).astype(F16)

    stat = np.zeros((128, KT * 48), dtype=F16)
    for kt in range(KT):
        rows = slice(kt * 128, (kt + 1) * 128)
        stat[:, kt * 48 : kt * 48 + 16] = sq[rows]
        stat[:, kt * 48 + 16 : kt * 48 + 32] = shlo[rows]
        stat[:, kt * 48 + 32 : kt * 48 + 48] = shhi[rows]

    # correction batch vectors from the ROUNDED stationaries (exact cancel)
    sum_sq = sq.astype(np.float64).sum(axis=0)      # [B]
    sum_shlo = shlo.astype(np.float64).sum(axis=0)
    sum_shhi = shhi.astype(np.float64).sum(axis=0)
    rowsum = inp64.sum(axis=1)                      # [B]
    sq_h, sq_l = _split_hi_lo(sum_sq)
    slo_h, slo_l = _split_hi_lo(sum_shlo)
    shi_h, shi_l = _split_hi_lo(sum_shhi)
    rs_h, rs_l = _split_hi_lo(rowsum)
    corrL = np.zeros((KC, 16), dtype=F16)
    corrL[0], corrL[1] = sq_h, sq_l
    corrL[2], corrL[3] = slo_h, slo_l
    corrL[4], corrL[5] = shi_h, shi_l
    corrL[6], corrL[7] = rs_h, rs_h
    corrL[8] = rs_l

    qw = np.asarray(quant_weight)
    scales = np.asarray(scales, dtype=np.float64).reshape(-1)
    zeros = np.asarray(zeros, dtype=np.float64).reshape(-1)

    stat_u16 = stat.view(np.uint16)
    in_maps = []
    for cidx in range(NCORES):
        rows = slice(cidx * OS, (cidx + 1) * OS)
        qc = qw[rows].astype(np.uint8).T  # [HALF, OS] natural columns
        # byte-pair columns (m, 688+m) -> uint16 elements
        qc2 = np.empty((HALF, OS), dtype=np.uint8)
        qc2[:, 0::2] = qc[:, :HOS]
        qc2[:, 1::2] = qc[:, HOS:]
        qu16 = np.ascontiguousarray(qc2).view(np.uint16)  # [HALF, HOS]
        # regroup rows: q_all[r, kt*HOS + m] = qu16[kt*128 + r, m]
        q_all = np.ascontiguousarray(
            qu16.reshape(KT, 128, HOS).transpose(1, 0, 2).reshape(128, KT * HOS)
        )
        d0_c = np.concatenate(
            [q_all[:, 0:HOS], stat_u16[:, 0 : NS0 * 48]], axis=1
        )
        d1_c = np.concatenate(
            [stat_u16[:, NS0 * 48 :], q_all[:, HOS : 3 * HOS]], axis=1
        )
        q_c = np.ascontiguousarray(q_all[:, 3 * HOS :])

        z = zeros[rows]
        z_h, z_l = _split_hi_lo(z)
        corr_c = np.zeros((KC, 16 + OS), dtype=F16)
        corr_c[:, 0:16] = corrL
        corrR = corr_c[:, 16:]
        corrR[0] = -128.0
        corrR[1] = -128.0
        corrR[2, :HOS] = -128.0
        corrR[3, :HOS] = -128.0
        corrR[4, HOS:] = -128.0
        corrR[5, HOS:] = -128.0
        corrR[6] = -z_h
        corrR[7] = -z_l
        corrR[8] = -z_h
        sc_c = np.broadcast_to(scales[rows].astype(np.float32), (B, OS))
        csc_c = np.zeros((B, 16 + OS + 2 * OS), dtype=np.uint16)
        csc_c[0:KC, 0 : 16 + OS] = corr_c.view(np.uint16)
        csc_c[:, 16 + OS :] = np.ascontiguousarray(sc_c).view(np.uint16)
        in_maps.append(
            {
                "d0": d0_c,
                "d1": d1_c,
                "q": q_c,
                "csc": csc_c,
            }
        )
    return in_maps


def kernel(inp, quant_weight, scales, zeros):
    from concourse.bass_utils import run_bass_kernel_spmd

    nc = _get_program()
    in_maps = _host_prep(inp, quant_weight, scales, zeros)
    res = run_bass_kernel_spmd(nc, in_maps, core_ids=list(range(NCORES)))
    out = np.concatenate(
        [res.results[c]["out"] for c in range(NCORES)], axis=1
    )
    return np.ascontiguousarray(out.astype(np.float32))


# revision 30
# speedup vs baseline: 1.2521x; 1.0594x over previous
"""4-bit column-block-quantized linear (ColBlockQuantizedLinear) on 8 TRN2 cores.

Math:  out[b,o] = scales[o] * (sum_i inp[b,i]*wq[o,i] - zeros[o]*rowsum[b])
with packed bytes q[j,o] (j = i//2): low nibble l = wq[o,2j], high nibble
h = wq[o,2j+1].  Identity: sum_j a_j*l_j + b_j*h_j = sum_j a_j*q_j + c_j*h_j
with c = b - 16a, q = 16h + l.

Device scheme (fp16 bit-trick): the fp16 bit pattern 0x5800|x encodes the
value 128 + x/8 EXACTLY for any 8-bit x.  So each weight stream is ONE
dual-op DVE tensor_scalar pass over the packed u16 data:
    Qlo = (q16 & 0x00FF) | 0x5800   -> 128 + q_lo/8      (pairs with 8a)
    Qhi = (q16 >>  8)    | 0x5800   -> 128 + q_hi/8      (pairs with 8a)
    Hlo = (q16 & 0x00F0) | 0x5800   -> 128 + 2*h_lo      (pairs with c/2)
    Hhi = (q16 >> 12)    | 0x5800   -> 128 + h_hi/8      (pairs with 8c)
The +128 offsets cancel exactly against rank-1 rows built from the SAME
fp16-rounded stationaries, folded with -zeros*rowsum into a K=9 fp16 hi/lo
correction matmul issued LAST (start=True sits on the first stream matmuls;
corrections run warm and overlap the per-block scale/DMA tail).  Stationary
activation factors are single fp16.  Scales are applied on-device by
per-psum-block DVE tensor_tensor multiplies.

DMA economics (measured): a DMA instruction costs ~1.5-2us of queue overhead
plus ~one packet per SBUF partition row (~20-27ns each, FIFO per queue), so
the weight load is split into exactly FIVE single DMAs, one per engine queue
(sync/scalar/gpsimd/vector/tensor), each first-and-alone on its queue, sized
in need order.  kt0 and its stationaries ride the first sync DMA (fusion is
free - packets are per row); the remaining stationaries ride the scalar DMA.
All weight tiles stay resident in SBUF (~24KB/partition).  DVE unpack runs
in kt-pair slices over the resident tiles.  Seven zero matmuls on a memset
tile warm the PE (HAM un-throttle needs ~3.4us of sustained activity) so
the stream matmuls start at 2.4GHz.

Sharding: column-parallel over out_features (1376 rows/core), inputs
replicated; per-core output [16,1376] gathered on host.
"""

import numpy as np

B = 16
I = 4096
O = 11008
NCORES = 8
OS = O // NCORES          # 1376 out-features per core
HOS = OS // 2             # 688 packed u16 columns
HALF = I // 2             # 2048 packed (contraction) rows
KT = HALF // 128          # 16 contraction tiles
KC = 9                    # correction matmul contraction size
NDUMMY = 6                # PE warmup matmuls
SPL = 512                 # kt0 column split (psum-block boundary)
NS0 = 4                   # kt tiles whose stationaries ride DMA d0
# weight pair-DMAs after d0/d1: (first kt, n kt) -> round-robin queues in
# need order (gpsimd, sync, scalar, gpsimd, sync, scalar, gpsimd)
PAIRS = [(3, 2), (5, 2), (7, 2), (9, 2), (11, 2), (13, 2), (15, 1)]
# DVE/compute sub-chunks: (first kt, n kt)
CCHUNKS = [(1, 2), (3, 2), (5, 2), (7, 2), (9, 2), (11, 2), (13, 2), (15, 1)]
# psum o-blocks (each within one 688-column half, <=512 cols per fp32 bank)
BLKS = [(0, 512), (512, 176), (688, 512), (1200, 176)]

F16 = np.float16

_CACHE = {}


def _build_program():
    import concourse.bacc as bacc
    import concourse.mybir as mybir
    import concourse.tile as tile

    dt = mybir.dt
    op = mybir.AluOpType
    nc = bacc.Bacc("TRN2", target_bir_lowering=False)

    # d0 = [kt0 | stat(kt0..NS0)]; d1 = [stat(rest) | kt1-2]; d2..d4 = kt runs
    d0 = nc.dram_tensor("d0", [128, HOS + NS0 * 48], dt.uint16, kind="ExternalInput")
    d1 = nc.dram_tensor(
        "d1", [128, (KT - NS0) * 48 + 2 * HOS], dt.uint16, kind="ExternalInput"
    )
    q = nc.dram_tensor("q", [128, 13 * HOS], dt.uint16, kind="ExternalInput")
    csc = nc.dram_tensor(
        "csc", [B, (16 + OS) + 2 * OS], dt.uint16, kind="ExternalInput"
    )
    out = nc.dram_tensor("out", [B, OS], dt.float32, kind="ExternalOutput")

    with tile.TileContext(nc) as tc:
        with (
            tc.tile_pool(name="consts", bufs=1) as cpool,
            tc.tile_pool(name="wp", bufs=3) as wpool,
            tc.tile_pool(name="op", bufs=1) as opool,
            tc.tile_pool(name="ps", bufs=1, space="PSUM") as pspool,
        ):
            # PE warmup: zero matmuls while DMAs/DVE fill the pipeline
            dummy = cpool.tile([128, 512], dt.float16, name="dummy")
            ps_w = pspool.tile([16, 512], dt.float32, name="ps_w")
            nc.vector.memset(dummy, 0.0)
            for _ in range(NDUMMY):
                nc.tensor.matmul(
                    ps_w, dummy[:, 0:16], dummy, start=True, stop=True,
                    skip_group_check=True,
                )

            t0 = cpool.tile([128, HOS + NS0 * 48], dt.uint16, name="t0")
            nc.sync.dma_start(t0, d0[:, :])
            t1 = cpool.tile(
                [128, (KT - NS0) * 48 + 2 * HOS], dt.uint16, name="t1"
            )
            nc.scalar.dma_start(t1, d1[:, :])
            csc_sb = cpool.tile([B, 16 + OS + 2 * OS], dt.uint16, name="csc_sb")
            nc.scalar.dma_start(csc_sb, csc[:, :])

            QRR = [nc.gpsimd, nc.sync, nc.scalar]
            ptile = {}
            for pi, (k0, cw) in enumerate(PAIRS):
                pt = cpool.tile([128, cw * HOS], dt.uint16, name=f"pt{pi}")
                QRR[pi % 3].dma_start(
                    pt, q[:, (k0 - 3) * HOS : (k0 - 3 + cw) * HOS]
                )
                ptile[k0] = pt
            corr16 = csc_sb.bitcast(dt.float16)
            corrL_sb = corr16[0:KC, 0:16]
            corrR_sb = corr16[0:KC, 16 : 16 + OS]
            sc_sb = csc_sb.bitcast(dt.float32)[:, (16 + OS) // 2 : (16 + OS) // 2 + OS]

            st0_16 = t0.bitcast(dt.float16)  # stat kt0..NS0-1 at cols HOS+
            st1_16 = t1.bitcast(dt.float16)  # stat NS0..KT-1 at cols 0+

            def stat_slices(kt):
                if kt < NS0:
                    sb = HOS + kt * 48
                    src = st0_16
                else:
                    sb = (kt - NS0) * 48
                    src = st1_16
                return (
                    src[:, sb : sb + 16],
                    src[:, sb + 16 : sb + 32],
                    src[:, sb + 32 : sb + 48],
                )

            # source tile + column offset for each kt's packed data
            def kt_src(kt):
                if kt == 0:
                    return t0, 0
                if kt <= 2:
                    return t1, (KT - NS0) * 48 + (kt - 1) * HOS
                return ptile[kt], 0

            psums = [
                pspool.tile([B, n], dt.float32, name=f"ps{i}")
                for i, (s, n) in enumerate(BLKS)
            ]

            def unpack(dst4, src_ap):
                nc.vector.tensor_scalar(
                    dst4[0], src_ap, 0x00FF, 0x5800, op.bitwise_and, op.bitwise_or
                )
                nc.vector.tensor_scalar(
                    dst4[1], src_ap, 8, 0x5800, op.logical_shift_right, op.bitwise_or
                )
                nc.vector.tensor_scalar(
                    dst4[2], src_ap, 0x00F0, 0x5800, op.bitwise_and, op.bitwise_or
                )
                nc.vector.tensor_scalar(
                    dst4[3], src_ap, 12, 0x5800, op.logical_shift_right, op.bitwise_or
                )

            def kt_matmuls(kt, st4, off, first, last=False):
                sq, shlo, shhi = stat_slices(kt)
                qlo16 = st4[0].bitcast(dt.float16)
                qhi16 = st4[1].bitcast(dt.float16)
                hlo16 = st4[2].bitcast(dt.float16)
                hhi16 = st4[3].bitcast(dt.float16)
                for i, (s, n) in enumerate(BLKS):
                    if s < HOS:
                        a, b_ = off + s, off + s + n
                        nc.tensor.matmul(
                            psums[i], sq, qlo16[:, a:b_], start=first, stop=False
                        )
                        nc.tensor.matmul(
                            psums[i], shlo, hlo16[:, a:b_], start=False, stop=last
                        )
                    else:
                        a, b_ = off + s - HOS, off + s - HOS + n
                        nc.tensor.matmul(
                            psums[i], sq, qhi16[:, a:b_], start=first, stop=False
                        )
                        nc.tensor.matmul(
                            psums[i], shhi, hhi16[:, a:b_], start=False, stop=last
                        )

            # --- kt0 in two column pieces aligned to psum-block boundaries:
            # piece A (cols 0..511) feeds blocks 0,2; piece B feeds 1,3
            nB = HOS - SPL
            stA = [cpool.tile([128, SPL], dt.uint16, name=f"sA{k}") for k in range(4)]
            stB = [cpool.tile([128, nB], dt.uint16, name=f"sB{k}") for k in range(4)]
            unpack(stA, t0[:, 0:SPL])
            unpack(stB, t0[:, SPL:HOS])
            sq0, shlo0, shhi0 = stat_slices(0)
            nc.tensor.matmul(
                psums[0], sq0, stA[0].bitcast(dt.float16), start=True, stop=False
            )
            nc.tensor.matmul(
                psums[0], shlo0, stA[2].bitcast(dt.float16), start=False, stop=False
            )
            nc.tensor.matmul(
                psums[2], sq0, stA[1].bitcast(dt.float16), start=True, stop=False
            )
            nc.tensor.matmul(
                psums[2], shhi0, stA[3].bitcast(dt.float16), start=False, stop=False
            )
            nc.tensor.matmul(
                psums[1], sq0, stB[0].bitcast(dt.float16), start=True, stop=False
            )
            nc.tensor.matmul(
                psums[1], shlo0, stB[2].bitcast(dt.float16), start=False, stop=False
            )
            nc.tensor.matmul(
                psums[3], sq0, stB[1].bitcast(dt.float16), start=True, stop=False
            )
            nc.tensor.matmul(
                psums[3], shhi0, stB[3].bitcast(dt.float16), start=False, stop=False
            )

            # --- kt1..15 in pair sub-chunks over the resident tiles; the
            # corrections (-128*sum(coef), -zeros*rowsum) slot in mid-phase,
            # off the tail (accumulation order is free)
            for pi, (k0, cw) in enumerate(CCHUNKS):
                w = cw * HOS
                src, soff = kt_src(k0)
                st4 = [
                    wpool.tile([128, w], dt.uint16, name=f"s{pi}_{k}", tag=f"s{k}_{cw}")
                    for k in range(4)
                ]
                unpack(st4, src[:, soff : soff + w])
                for h in range(cw):
                    kt = k0 + h
                    kt_matmuls(kt, st4, h * HOS, False, last=(kt == KT - 1))
                if k0 == 5:
                    for i, (s, n) in enumerate(BLKS):
                        nc.tensor.matmul(
                            psums[i], corrL_sb, corrR_sb[:, s : s + n],
                            start=False, stop=False,
                        )

            o = opool.tile([B, OS], dt.float32, name="o")
            for i, (s, n) in enumerate(BLKS):
                nc.vector.tensor_tensor(
                    o[:, s : s + n], psums[i], sc_sb[:, s : s + n], op.mult
                )
            nc.gpsimd.dma_start(out[:, 0:HOS], o[:, 0:HOS])
            nc.scalar.dma_start(out[:, HOS:OS], o[:, HOS:OS])

    nc.finalize()
    return nc


def _get_program():
    if "nc" not in _CACHE:
        _CACHE["nc"] = _build_program()
    return _CACHE["nc"]


def _split_hi_lo(x64):
    hi = x64.astype(F16)
    lo = (x64 - hi.astype(np.float64)).astype(F16)
    return hi, lo


def _host_prep(inp, quant_weight, scales, zeros):
    """Per-core input maps: layout/precision prep only, no O(O*I) math."""
    inp64 = np.asarray(inp, dtype=np.float64)
    a = inp64[:, 0::2].T  # [HALF, B] even-i activations (pair with l / q)
    b = inp64[:, 1::2].T  # [HALF, B] odd-i activations (pair with h)
    c = b - 16.0 * a

    sq = (8.0 * a).astype(F16)      # [HALF, B]
    shlo = (c / 2.0).astype(F16)
    shhi = (8.0 * c).astype(F16)

    stat = np.zeros((128, KT * 48), dtype=F16)
    for kt in range(KT):
        rows = slice(kt * 128, (kt + 1) * 128)
        stat[:, kt * 48 : kt * 48 + 16] = sq[rows]
        stat[:, kt * 48 + 16 : kt * 48 + 32] = shlo[rows]
        stat[:, kt * 48 + 32 : kt * 48 + 48] = shhi[rows]

    # correction batch vectors from the ROUNDED stationaries (exact cancel)
    sum_sq = sq.astype(np.float64).sum(axis=0)      # [B]
    sum_shlo = shlo.astype(np.float64).sum(axis=0)
    sum_shhi = shhi.astype(np.float64).sum(axis=0)
    rowsum = inp64.sum(axis=1)                      # [B]
    sq_h, sq_l = _split_hi_lo(sum_sq)
    slo_h, slo_l = _split_hi_lo(sum_shlo)
    shi_h, shi_l = _split_hi_lo(sum_shhi)
    rs_h, rs_l = _split_hi_lo(rowsum)
    corrL = np.zeros((KC, 16), dtype=F16)
    corrL[0], corrL[1] = sq_h, sq_l
    corrL[2], corrL[3] = slo_h, slo_l
    corrL[4], corrL[5] = shi_h, shi_l
    corrL[6], corrL[7] = rs_h, rs_h
    corrL[8] = rs_l

    qw = np.asarray(quant_weight)
    scales = np.asarray(scales, dtype=np.float64).reshape(-1)
    zeros = np.asarray(zeros, dtype=np.float64).reshape(-1)

    stat_u16 = stat.view(np.uint16)
    in_maps = []
    for cidx in range(NCORES):
        rows = slice(cidx * OS, (cidx + 1) * OS)
        qc = qw[rows].astype(np.uint8).T  # [HALF, OS] natural columns
        # byte-pair columns (m, 688+m) -> uint16 elements
        qc2 = np.empty((HALF, OS), dtype=np.uint8)
        qc2[:, 0::2] = qc[:, :HOS]
        qc2[:, 1::2] = qc[:, HOS:]
        qu16 = np.ascontiguousarray(qc2).view(np.uint16)  # [HALF, HOS]
        # regroup rows: q_all[r, kt*HOS + m] = qu16[kt*128 + r, m]
        q_all = np.ascontiguousarray(
            qu16.reshape(KT, 128, HOS).transpose(1, 0, 2).reshape(128, KT * HOS)
        )
        d0_c = np.concatenate(
            [q_all[:, 0:HOS], stat_u16[:, 0 : NS0 * 48]], axis=1
        )
        d1_c = np.concatenate(
            [stat_u16[:, NS0 * 48 :], q_all[:, HOS : 3 * HOS]], axis=1
        )
        q_c = np.ascontiguousarray(q_all[:, 3 * HOS :])

        z = zeros[rows]
        z_h, z_l = _split_hi_lo(z)
        corr_c = np.zeros((KC, 16 + OS), dtype=F16)
        corr_c[:, 0:16] = corrL
        corrR = corr_c[:, 16:]
        corrR[0] = -128.0
        corrR[1] = -128.0
        corrR[2, :HOS] = -128.0
        corrR[3, :HOS] = -128.0
        corrR[4, HOS:] = -128.0
        corrR[5, HOS:] = -128.0
        corrR[6] = -z_h
        corrR[7] = -z_l
        corrR[8] = -z_h
        sc_c = np.broadcast_to(scales[rows].astype(np.float32), (B, OS))
        csc_c = np.zeros((B, 16 + OS + 2 * OS), dtype=np.uint16)
        csc_c[0:KC, 0 : 16 + OS] = corr_c.view(np.uint16)
        csc_c[:, 16 + OS :] = np.ascontiguousarray(sc_c).view(np.uint16)
        in_maps.append(
            {
                "d0": d0_c,
                "d1": d1_c,
                "q": q_c,
                "csc": csc_c,
            }
        )
    return in_maps


def kernel(inp, quant_weight, scales, zeros):
    from concourse.bass_utils import run_bass_kernel_spmd

    nc = _get_program()
    in_maps = _host_prep(inp, quant_weight, scales, zeros)
    res = run_bass_kernel_spmd(nc, in_maps, core_ids=list(range(NCORES)))
    out = np.concatenate(
        [res.results[c]["out"] for c in range(NCORES)], axis=1
    )
    return np.ascontiguousarray(out.astype(np.float32))
